# revision 5
# baseline (speedup 1.0000x reference)
"""Trainium2 Bass kernel for nn_CoupledOscillatorNetwork.

Math: each inner step of the reference is affine in the flattened state
s = reshape(y, [B, 1058]) (2-channel field on a 23x23 torus):

    v' = dt_l*(C - g*I) x + ((1 - dt_l*a) I + dt_l*R) v + dt_l*c0
    x' = x + dt_l * v'

with C, R the circular 3x3 conv matrices. Ten inner steps therefore
collapse into ONE dense affine map s -> M s + d with M = A^10 computed on
the host in float64 from the (tiny) parameter tensors. The device only
runs the outer recurrence: s_{t+1} = M_aug s_t on an augmented
(homogeneous) state, writing every state to DRAM. Pure data parallelism:
batch 1024 is sharded 128 per NeuronCore across 8 cores.

Device layout (per core), state-major:
  S [1152 x 128]  (state padded 1059->1152 = 9 chunks of 128, batch=128 free)
  per outer step, per output chunk mc: PSUM[128,128] accumulates
  9 matmuls  M_pad^T[kc-chunk, mc-cols] . S[kc-chunk]  ->  copy to next
  state tile + DMA to DRAM.
"""

import numpy as np
from contextlib import ExitStack

import concourse.bass as bass
import concourse.bacc as bacc
import concourse.mybir as mybir
import concourse.tile as tile
from concourse.bass_utils import run_bass_kernel_spmd

SPATIAL = 23
P2 = SPATIAL * SPATIAL          # 529
D = 2 * P2                      # 1058
NK = 9                          # state chunks
DPAD = NK * 128                 # 1152 (state padded incl. homogeneous row 1058)
NCORES = 8
BLOC = 128                      # batch per core

# ---------------------------------------------------------------- host math

def _conv_matrix(W):
    W = np.asarray(W, np.float64).reshape(3, 3)
    idx = np.arange(P2).reshape(SPATIAL, SPATIAL)
    C = np.zeros((P2, P2))
    rows = np.arange(P2)
    for di in range(3):
        for dj in range(3):
            src = np.roll(np.roll(idx, -(di - 1), axis=0), -(dj - 1), axis=1)
            C[rows, src.ravel()] += W[di, dj]
    return C


def _build_step_map(W_coupling, b_coupling, W_resid, b_resid, b_bar, dt, alpha, gamma):
    dt_l = 1.0 / (1.0 + np.exp(-np.float64(dt)))
    gamma_p = max(float(gamma), 0.0)
    alpha_p = max(float(alpha), 0.0)
    C = _conv_matrix(W_coupling)
    R = _conv_matrix(W_resid)
    I = np.eye(P2)
    c0 = (float(np.asarray(b_coupling).ravel()[0])
          + float(np.asarray(b_resid).ravel()[0])
          + np.asarray(b_bar, np.float64).ravel())
    A_vx = dt_l * (C - gamma_p * I)
    A_vv = (1.0 - dt_l * alpha_p) * I + dt_l * R
    A = np.zeros((D, D))
    A[0::2, 0::2] = I + dt_l * A_vx
    A[0::2, 1::2] = dt_l * A_vv
    A[1::2, 0::2] = A_vx
    A[1::2, 1::2] = A_vv
    b = np.zeros(D)
    b[0::2] = dt_l * dt_l * c0
    b[1::2] = dt_l * c0
    return A, b


def _collapse(A, b, k):
    M = np.eye(A.shape[0])
    d = np.zeros(A.shape[0])
    for _ in range(k):
        M = A @ M
        d = A @ d + b
    return M, d


def _augment_pad(M, d):
    """[DPAD, DPAD] fp64 with homogeneous (bias) row at index D."""
    Mp = np.zeros((DPAD, DPAD))
    Mp[:D, :D] = M
    Mp[:D, D] = d
    Mp[D, D] = 1.0
    return Mp


def _mt_host(Mp, np_dtype=np.float32):
    """lhsT layout: mt[p, kc, m] = Mp[m, kc*128+p]."""
    return np.ascontiguousarray(
        Mp.T.reshape(NK, 128, DPAD).transpose(1, 0, 2)).astype(np_dtype)


# ---------------------------------------------------------------- device IR

_prog_cache = {}


def _build_program(T):
    """Sequential fp32 recurrence: T outer steps, one matmul group per chunk."""
    key = ("v1", T)
    if key in _prog_cache:
        return _prog_cache[key]

    nc = bacc.Bacc("TRN2")
    f32 = mybir.dt.float32
    mt_d = nc.dram_tensor("mt", [128, NK, DPAD], f32, kind="ExternalInput")
    s0_d = nc.dram_tensor("s0", [128, NK, BLOC], f32, kind="ExternalInput")
    y_d = nc.dram_tensor("y", [T, D, BLOC], f32, kind="ExternalOutput")

    with tile.TileContext(nc) as tc, ExitStack() as ctx:
        const = ctx.enter_context(tc.tile_pool(name="const", bufs=1))
        state = ctx.enter_context(tc.tile_pool(name="state", bufs=2))
        psum = ctx.enter_context(tc.tile_pool(name="psum", bufs=4, space="PSUM"))

        mt_sb = const.tile([128, NK, DPAD], f32)
        nc.sync.dma_start(mt_sb[:], mt_d[:])
        s_cur = state.tile([128, NK, BLOC], f32, tag="st")
        nc.sync.dma_start(s_cur[:], s0_d[:])
        # Collapse the many DMA-queue completion semaphores into one barrier
        # so the first matmuls don't exceed the per-instruction wait limit.
        tc.strict_bb_all_engine_barrier()

        for t in range(T):
            s_next = state.tile([128, NK, BLOC], f32, tag="st")
            for mc in range(NK):
                ps = psum.tile([128, BLOC], mybir.dt.float32, tag="ps")
                for kc in range(NK):
                    nc.tensor.matmul(
                        ps,
                        mt_sb[:, kc, mc * 128:(mc + 1) * 128],
                        s_cur[:, kc, :],
                        start=(kc == 0), stop=(kc == NK - 1))
                nc.vector.tensor_copy(s_next[:, mc, :], ps)
                if mc < NK - 1:
                    nc.sync.dma_start(y_d[t, mc * 128:(mc + 1) * 128, :],
                                      s_next[:, mc, :])
                else:
                    nc.sync.dma_start(y_d[t, 8 * 128:D, :],
                                      s_next[:D - 8 * 128, mc, :])
            s_cur = s_next

    nc.finalize()
    _prog_cache[key] = nc
    return nc


def _build_program_chained(T, mm_dt=None):
    """4 interleaved chains (t mod 4) so the PE free dim is 512, where
    fp32r streams 1 cycle/row instead of fp32's 4.

    Ramp (on device): s1 = M s0 ; [s2|s3] = M^2 [s0|s1].
    Steady: U_r = M^4 U_{r-1} with U holding 4 states side by side.
    Requires T >= 4."""
    mm_dt = mm_dt or mybir.dt.float32r
    key = ("v2", T, mm_dt)
    if key in _prog_cache:
        return _prog_cache[key]

    q_full = (T - 3) // 4            # steady rounds: r=1..q_full -> t=4r..4r+3
    tr = T - (4 * q_full + 3)        # 0..3 tail states

    nc = bacc.Bacc("TRN2")
    f32 = mybir.dt.float32
    mt1_d = nc.dram_tensor("mt1", [128, NK, DPAD], f32, kind="ExternalInput")
    mt2_d = nc.dram_tensor("mt2", [128, NK, DPAD], f32, kind="ExternalInput")
    mt4_d = nc.dram_tensor("mt4", [128, NK, DPAD], f32, kind="ExternalInput")
    s0_d = nc.dram_tensor("s0", [128, NK, BLOC], f32, kind="ExternalInput")
    y_d = nc.dram_tensor("y", [T, D, BLOC], f32, kind="ExternalOutput")

    with tile.TileContext(nc) as tc, ExitStack() as ctx:
        const = ctx.enter_context(tc.tile_pool(name="const", bufs=1))
        state = ctx.enter_context(tc.tile_pool(name="state", bufs=2))
        psum = ctx.enter_context(tc.tile_pool(name="psum", bufs=4, space="PSUM"))

        mt1_sb = const.tile([128, NK, DPAD], f32)
        mt2_sb = const.tile([128, NK, DPAD], f32)
        mt4_sb = const.tile([128, NK, DPAD], f32)
        nc.sync.dma_start(mt1_sb[:], mt1_d[:])
        nc.sync.dma_start(mt2_sb[:], mt2_d[:])
        nc.sync.dma_start(mt4_sb[:], mt4_d[:])
        u_cur = state.tile([128, NK, 4 * BLOC], f32, tag="st")
        nc.sync.dma_start(u_cur[:, :, 0:BLOC], s0_d[:])
        tc.strict_bb_all_engine_barrier()

        def mm(ps, mt_sb, kc, mc, rhs):
            nc.tensor.matmul(
                ps,
                mt_sb[:, kc, mc * 128:(mc + 1) * 128].bitcast(mm_dt),
                rhs.bitcast(mm_dt),
                start=(kc == 0), stop=(kc == NK - 1))

        def emit(t, mc, src_cols):
            if mc < NK - 1:
                nc.sync.dma_start(y_d[t, mc * 128:(mc + 1) * 128, :], src_cols)
            else:
                nc.sync.dma_start(y_d[t, 8 * 128:D, :], src_cols[:D - 8 * 128, :])

        # ramp 1: s1 -> u cols [1B:2B)
        for mc in range(NK):
            ps = psum.tile([128, BLOC], f32, tag="ps")
            for kc in range(NK):
                mm(ps, mt1_sb, kc, mc, u_cur[:, kc, 0:BLOC])
            nc.vector.tensor_copy(u_cur[:, mc, BLOC:2 * BLOC], ps)
            emit(1, mc, u_cur[:, mc, BLOC:2 * BLOC])
        # ramp 2: [s2|s3] -> u cols [2B:4B)
        for mc in range(NK):
            ps = psum.tile([128, 2 * BLOC], f32, tag="ps")
            for kc in range(NK):
                mm(ps, mt2_sb, kc, mc, u_cur[:, kc, 0:2 * BLOC])
            nc.vector.tensor_copy(u_cur[:, mc, 2 * BLOC:4 * BLOC], ps)
            emit(2, mc, u_cur[:, mc, 2 * BLOC:3 * BLOC])
            emit(3, mc, u_cur[:, mc, 3 * BLOC:4 * BLOC])
        # steady
        for r in range(1, q_full + 1):
            u_next = state.tile([128, NK, 4 * BLOC], f32, tag="st")
            for mc in range(NK):
                ps = psum.tile([128, 4 * BLOC], f32, tag="ps")
                for kc in range(NK):
                    mm(ps, mt4_sb, kc, mc, u_cur[:, kc, :])
                nc.vector.tensor_copy(u_next[:, mc, :], ps)
                for c in range(4):
                    emit(4 * r + c, mc, u_next[:, mc, c * BLOC:(c + 1) * BLOC])
            u_cur = u_next
        # tail
        if tr:
            sc = state.tile([128, NK, 4 * BLOC], f32, tag="st")
            for mc in range(NK):
                ps = psum.tile([128, tr * BLOC], f32, tag="ps")
                for kc in range(NK):
                    mm(ps, mt4_sb, kc, mc, u_cur[:, kc, 0:tr * BLOC])
                nc.vector.tensor_copy(sc[:, mc, 0:tr * BLOC], ps)
                for c in range(tr):
                    emit(4 * (q_full + 1) + c, mc, sc[:, mc, c * BLOC:(c + 1) * BLOC])

    nc.finalize()
    _prog_cache[key] = nc
    return nc


# ---------------------------------------------------------------- entry

VARIANT = "v1"
LAST_RESULTS = None


def kernel(**inputs):
    y0 = np.ascontiguousarray(np.asarray(inputs["y0"], np.float32))
    T = int(np.asarray(inputs["num_steps_forward"]))
    B = y0.shape[0]
    assert y0.shape == (B, D) and B == NCORES * BLOC

    out = np.empty((B, T + 1, D), np.float32)
    out[:, 0, :] = y0
    if T == 0:
        return out

    A, b = _build_step_map(
        inputs["W_coupling"], inputs["b_coupling"], inputs["W_resid"],
        inputs["b_resid"], inputs["b_bar"], inputs["dt"], inputs["alpha"],
        inputs["gamma"])
    M, d = _collapse(A, b, 10)
    mt = _mt_host(_augment_pad(M, d))

    # s0 per core: s0[p, kc, b] = s_pad[kc*128+p, b]
    in_maps = []
    for c in range(NCORES):
        sp = np.zeros((DPAD, BLOC), np.float32)
        sp[:D] = y0[c * BLOC:(c + 1) * BLOC].T
        sp[D] = 1.0
        s0c = np.ascontiguousarray(sp.reshape(NK, 128, BLOC).transpose(1, 0, 2))
        in_maps.append({"mt": mt, "s0": s0c})

    nc = _build_program(T)
    global LAST_RESULTS
    LAST_RESULTS = run_bass_kernel_spmd(nc, in_maps, core_ids=list(range(NCORES)))
    for c in range(NCORES):
        yc = LAST_RESULTS.results[c]["y"]            # [T, D, BLOC]
        out[c * BLOC:(c + 1) * BLOC, 1:, :] = yc.transpose(2, 0, 1)
    return out


# revision 9
# speedup vs baseline: 2.3244x; 2.3244x over previous
"""Trainium2 Bass kernel for nn_CoupledOscillatorNetwork.

Math: each inner step of the reference is affine in the flattened state
s = reshape(y, [B, 1058]) (2-channel field on a 23x23 torus):

    v' = dt_l*(C - g*I) x + ((1 - dt_l*a) I + dt_l*R) v + dt_l*c0
    x' = x + dt_l * v'

with C, R the circular 3x3 conv matrices. Ten inner steps therefore
collapse into ONE dense affine map s -> M s + d with M = A^10 computed on
the host in float64 from the (tiny) parameter tensors. The device only
runs the outer recurrence: s_{t+1} = M_aug s_t on an augmented
(homogeneous) state, writing every state to DRAM. Pure data parallelism:
batch 1024 is sharded 128 per NeuronCore across 8 cores.

Device layout (per core), state-major:
  S [1152 x 128]  (state padded 1059->1152 = 9 chunks of 128, batch=128 free)
  per outer step, per output chunk mc: PSUM[128,128] accumulates
  9 matmuls  M_pad^T[kc-chunk, mc-cols] . S[kc-chunk]  ->  copy to next
  state tile + DMA to DRAM.
"""

import numpy as np
from contextlib import ExitStack

import concourse.bass as bass
import concourse.bacc as bacc
import concourse.mybir as mybir
import concourse.tile as tile
from concourse.bass_utils import run_bass_kernel_spmd

SPATIAL = 23
P2 = SPATIAL * SPATIAL          # 529
D = 2 * P2                      # 1058
NK = 9                          # state chunks
DPAD = NK * 128                 # 1152 (state padded incl. homogeneous row 1058)
NCORES = 8
BLOC = 128                      # batch per core

# ---------------------------------------------------------------- host math

def _conv_matrix(W):
    W = np.asarray(W, np.float64).reshape(3, 3)
    idx = np.arange(P2).reshape(SPATIAL, SPATIAL)
    C = np.zeros((P2, P2))
    rows = np.arange(P2)
    for di in range(3):
        for dj in range(3):
            src = np.roll(np.roll(idx, -(di - 1), axis=0), -(dj - 1), axis=1)
            C[rows, src.ravel()] += W[di, dj]
    return C


def _build_step_map(W_coupling, b_coupling, W_resid, b_resid, b_bar, dt, alpha, gamma):
    dt_l = 1.0 / (1.0 + np.exp(-np.float64(dt)))
    gamma_p = max(float(gamma), 0.0)
    alpha_p = max(float(alpha), 0.0)
    C = _conv_matrix(W_coupling)
    R = _conv_matrix(W_resid)
    I = np.eye(P2)
    c0 = (float(np.asarray(b_coupling).ravel()[0])
          + float(np.asarray(b_resid).ravel()[0])
          + np.asarray(b_bar, np.float64).ravel())
    A_vx = dt_l * (C - gamma_p * I)
    A_vv = (1.0 - dt_l * alpha_p) * I + dt_l * R
    A = np.zeros((D, D))
    A[0::2, 0::2] = I + dt_l * A_vx
    A[0::2, 1::2] = dt_l * A_vv
    A[1::2, 0::2] = A_vx
    A[1::2, 1::2] = A_vv
    b = np.zeros(D)
    b[0::2] = dt_l * dt_l * c0
    b[1::2] = dt_l * c0
    return A, b


def _collapse(A, b, k):
    M = np.eye(A.shape[0])
    d = np.zeros(A.shape[0])
    for _ in range(k):
        M = A @ M
        d = A @ d + b
    return M, d


def _augment_pad(M, d):
    """[DPAD, DPAD] fp64 with homogeneous (bias) row at index D."""
    Mp = np.zeros((DPAD, DPAD))
    Mp[:D, :D] = M
    Mp[:D, D] = d
    Mp[D, D] = 1.0
    return Mp


def _mt_host(Mp, np_dtype=np.float32):
    """lhsT layout: mt[p, kc, m] = Mp[m, kc*128+p]."""
    return np.ascontiguousarray(
        Mp.T.reshape(NK, 128, DPAD).transpose(1, 0, 2)).astype(np_dtype)


# ---------------------------------------------------------------- device IR

_prog_cache = {}


def _build_program(T):
    """Sequential fp32 recurrence: T outer steps, one matmul group per chunk."""
    key = ("v1", T)
    if key in _prog_cache:
        return _prog_cache[key]

    nc = bacc.Bacc("TRN2")
    f32 = mybir.dt.float32
    mt_d = nc.dram_tensor("mt", [128, NK, DPAD], f32, kind="ExternalInput")
    s0_d = nc.dram_tensor("s0", [128, NK, BLOC], f32, kind="ExternalInput")
    y_d = nc.dram_tensor("y", [T, D, BLOC], f32, kind="ExternalOutput")

    with tile.TileContext(nc) as tc, ExitStack() as ctx:
        const = ctx.enter_context(tc.tile_pool(name="const", bufs=1))
        state = ctx.enter_context(tc.tile_pool(name="state", bufs=2))
        psum = ctx.enter_context(tc.tile_pool(name="psum", bufs=4, space="PSUM"))

        mt_sb = const.tile([128, NK, DPAD], f32)
        nc.sync.dma_start(mt_sb[:], mt_d[:])
        s_cur = state.tile([128, NK, BLOC], f32, tag="st")
        nc.sync.dma_start(s_cur[:], s0_d[:])
        # Collapse the many DMA-queue completion semaphores into one barrier
        # so the first matmuls don't exceed the per-instruction wait limit.
        tc.strict_bb_all_engine_barrier()

        for t in range(T):
            s_next = state.tile([128, NK, BLOC], f32, tag="st")
            for mc in range(NK):
                ps = psum.tile([128, BLOC], mybir.dt.float32, tag="ps")
                for kc in range(NK):
                    nc.tensor.matmul(
                        ps,
                        mt_sb[:, kc, mc * 128:(mc + 1) * 128],
                        s_cur[:, kc, :],
                        start=(kc == 0), stop=(kc == NK - 1))
                nc.vector.tensor_copy(s_next[:, mc, :], ps)
                if mc < NK - 1:
                    nc.sync.dma_start(y_d[t, mc * 128:(mc + 1) * 128, :],
                                      s_next[:, mc, :])
                else:
                    nc.sync.dma_start(y_d[t, 8 * 128:D, :],
                                      s_next[:D - 8 * 128, mc, :])
            s_cur = s_next

    nc.finalize()
    _prog_cache[key] = nc
    return nc


def _build_program_chained(T, mm_dt=None):
    """4 interleaved chains (t mod 4) so the PE free dim is 512, where
    fp32r streams 1 cycle/row instead of fp32's 4.

    Ramp (on device): s1 = M s0 ; [s2|s3] = M^2 [s0|s1].
    Steady: U_r = M^4 U_{r-1} with U holding 4 states side by side.
    Requires T >= 4."""
    mm_dt = mm_dt or mybir.dt.float32r
    key = ("v2", T, mm_dt)
    if key in _prog_cache:
        return _prog_cache[key]

    q_full = (T - 3) // 4            # steady rounds: r=1..q_full -> t=4r..4r+3
    tr = T - (4 * q_full + 3)        # 0..3 tail states

    nc = bacc.Bacc("TRN2")
    f32 = mybir.dt.float32
    mt1_d = nc.dram_tensor("mt1", [128, NK, DPAD], mm_dt, kind="ExternalInput")
    mt2_d = nc.dram_tensor("mt2", [128, NK, DPAD], mm_dt, kind="ExternalInput")
    mt4_d = nc.dram_tensor("mt4", [128, NK, DPAD], mm_dt, kind="ExternalInput")
    s0_d = nc.dram_tensor("s0", [128, NK, BLOC], mm_dt, kind="ExternalInput")
    y_d = nc.dram_tensor("y", [T, D, BLOC], f32, kind="ExternalOutput")

    with tile.TileContext(nc) as tc, ExitStack() as ctx:
        const = ctx.enter_context(tc.tile_pool(name="const", bufs=1))
        state = ctx.enter_context(tc.tile_pool(name="state", bufs=2))
        psum = ctx.enter_context(tc.tile_pool(name="psum", bufs=4, space="PSUM"))

        mt1_sb = const.tile([128, NK, DPAD], mm_dt)
        mt2_sb = const.tile([128, NK, DPAD], mm_dt)
        mt4_sb = const.tile([128, NK, DPAD], mm_dt)
        nc.sync.dma_start(mt1_sb[:], mt1_d[:])
        nc.sync.dma_start(mt2_sb[:], mt2_d[:])
        nc.sync.dma_start(mt4_sb[:], mt4_d[:])
        u_cur = state.tile([128, NK, 4 * BLOC], mm_dt, tag="st")
        nc.sync.dma_start(u_cur[:, :, 0:BLOC], s0_d[:])
        tc.strict_bb_all_engine_barrier()

        def mm(ps, mt_sb, kc, mc, rhs):
            nc.tensor.matmul(
                ps,
                mt_sb[:, kc, mc * 128:(mc + 1) * 128],
                rhs,
                start=(kc == 0), stop=(kc == NK - 1))

        def emit(t, mc, src_cols):
            # state t (1-based) lands at y_d[t-1]; bytes of f32r are f32
            src_cols = src_cols.bitcast(f32)
            if mc < NK - 1:
                nc.sync.dma_start(y_d[t - 1, mc * 128:(mc + 1) * 128, :], src_cols)
            else:
                nc.sync.dma_start(y_d[t - 1, 8 * 128:D, :], src_cols[:D - 8 * 128, :])

        # ramp 1: s1 -> u cols [1B:2B)
        for mc in range(NK):
            ps = psum.tile([128, BLOC], f32, tag="ps")
            for kc in range(NK):
                mm(ps, mt1_sb, kc, mc, u_cur[:, kc, 0:BLOC])
            nc.vector.tensor_copy(u_cur[:, mc, BLOC:2 * BLOC], ps)
            emit(1, mc, u_cur[:, mc, BLOC:2 * BLOC])
        # ramp 2: [s2|s3] -> u cols [2B:4B)
        for mc in range(NK):
            ps = psum.tile([128, 2 * BLOC], f32, tag="ps")
            for kc in range(NK):
                mm(ps, mt2_sb, kc, mc, u_cur[:, kc, 0:2 * BLOC])
            nc.vector.tensor_copy(u_cur[:, mc, 2 * BLOC:4 * BLOC], ps)
            emit(2, mc, u_cur[:, mc, 2 * BLOC:3 * BLOC])
            emit(3, mc, u_cur[:, mc, 3 * BLOC:4 * BLOC])
        # steady
        for r in range(1, q_full + 1):
            u_next = state.tile([128, NK, 4 * BLOC], mm_dt, tag="st")
            for mc in range(NK):
                ps = psum.tile([128, 4 * BLOC], f32, tag="ps")
                for kc in range(NK):
                    mm(ps, mt4_sb, kc, mc, u_cur[:, kc, :])
                nc.vector.tensor_copy(u_next[:, mc, :], ps)
                for c in range(4):
                    emit(4 * r + c, mc, u_next[:, mc, c * BLOC:(c + 1) * BLOC])
            u_cur = u_next
        # tail
        if tr:
            sc = state.tile([128, NK, 4 * BLOC], mm_dt, tag="st")
            for mc in range(NK):
                ps = psum.tile([128, tr * BLOC], f32, tag="ps")
                for kc in range(NK):
                    mm(ps, mt4_sb, kc, mc, u_cur[:, kc, 0:tr * BLOC])
                nc.vector.tensor_copy(sc[:, mc, 0:tr * BLOC], ps)
                for c in range(tr):
                    emit(4 * (q_full + 1) + c, mc, sc[:, mc, c * BLOC:(c + 1) * BLOC])

    nc.finalize()
    _prog_cache[key] = nc
    return nc


# ---------------------------------------------------------------- entry

VARIANT = "v2"
LAST_RESULTS = None


def kernel(**inputs):
    y0 = np.ascontiguousarray(np.asarray(inputs["y0"], np.float32))
    T = int(np.asarray(inputs["num_steps_forward"]))
    B = y0.shape[0]
    assert y0.shape == (B, D) and B == NCORES * BLOC

    out = np.empty((B, T + 1, D), np.float32)
    out[:, 0, :] = y0
    if T == 0:
        return out

    A, b = _build_step_map(
        inputs["W_coupling"], inputs["b_coupling"], inputs["W_resid"],
        inputs["b_resid"], inputs["b_bar"], inputs["dt"], inputs["alpha"],
        inputs["gamma"])
    M, d = _collapse(A, b, 10)
    Mp = _augment_pad(M, d)

    use_v2 = VARIANT == "v2" and T >= 4
    if use_v2:
        Mp2 = Mp @ Mp
        weights = {"mt1": _mt_host(Mp), "mt2": _mt_host(Mp2),
                   "mt4": _mt_host(Mp2 @ Mp2)}
        nc = _build_program_chained(T)
    else:
        weights = {"mt": _mt_host(Mp)}
        nc = _build_program(T)

    # s0 per core: s0[p, kc, b] = s_pad[kc*128+p, b]
    in_maps = []
    for c in range(NCORES):
        sp = np.zeros((DPAD, BLOC), np.float32)
        sp[:D] = y0[c * BLOC:(c + 1) * BLOC].T
        sp[D] = 1.0
        s0c = np.ascontiguousarray(sp.reshape(NK, 128, BLOC).transpose(1, 0, 2))
        in_maps.append({**weights, "s0": s0c})
    global LAST_RESULTS
    LAST_RESULTS = run_bass_kernel_spmd(nc, in_maps, core_ids=list(range(NCORES)))
    for c in range(NCORES):
        yc = LAST_RESULTS.results[c]["y"]            # [T, D, BLOC]
        out[c * BLOC:(c + 1) * BLOC, 1:, :] = yc.transpose(2, 0, 1)
    return out


# revision 10
# speedup vs baseline: 2.3924x; 1.0292x over previous
"""Trainium2 Bass kernel for nn_CoupledOscillatorNetwork.

Math: each inner step of the reference is affine in the flattened state
s = reshape(y, [B, 1058]) (2-channel field on a 23x23 torus):

    v' = dt_l*(C - g*I) x + ((1 - dt_l*a) I + dt_l*R) v + dt_l*c0
    x' = x + dt_l * v'

with C, R the circular 3x3 conv matrices. Ten inner steps therefore
collapse into ONE dense affine map s -> M s + d with M = A^10 computed on
the host in float64 from the (tiny) parameter tensors. The device only
runs the outer recurrence: s_{t+1} = M_aug s_t on an augmented
(homogeneous) state, writing every state to DRAM. Pure data parallelism:
batch 1024 is sharded 128 per NeuronCore across 8 cores.

Device layout (per core), state-major:
  S [1152 x 128]  (state padded 1059->1152 = 9 chunks of 128, batch=128 free)
  per outer step, per output chunk mc: PSUM[128,128] accumulates
  9 matmuls  M_pad^T[kc-chunk, mc-cols] . S[kc-chunk]  ->  copy to next
  state tile + DMA to DRAM.
"""

import numpy as np
from contextlib import ExitStack

import concourse.bass as bass
import concourse.bacc as bacc
import concourse.mybir as mybir
import concourse.tile as tile
from concourse.bass_utils import run_bass_kernel_spmd

SPATIAL = 23
P2 = SPATIAL * SPATIAL          # 529
D = 2 * P2                      # 1058
NK = 9                          # state chunks
DPAD = NK * 128                 # 1152 (state padded incl. homogeneous row 1058)
NCORES = 8
BLOC = 128                      # batch per core

# ---------------------------------------------------------------- host math

def _conv_matrix(W):
    W = np.asarray(W, np.float64).reshape(3, 3)
    idx = np.arange(P2).reshape(SPATIAL, SPATIAL)
    C = np.zeros((P2, P2))
    rows = np.arange(P2)
    for di in range(3):
        for dj in range(3):
            src = np.roll(np.roll(idx, -(di - 1), axis=0), -(dj - 1), axis=1)
            C[rows, src.ravel()] += W[di, dj]
    return C


def _build_step_map(W_coupling, b_coupling, W_resid, b_resid, b_bar, dt, alpha, gamma):
    dt_l = 1.0 / (1.0 + np.exp(-np.float64(dt)))
    gamma_p = max(float(gamma), 0.0)
    alpha_p = max(float(alpha), 0.0)
    C = _conv_matrix(W_coupling)
    R = _conv_matrix(W_resid)
    I = np.eye(P2)
    c0 = (float(np.asarray(b_coupling).ravel()[0])
          + float(np.asarray(b_resid).ravel()[0])
          + np.asarray(b_bar, np.float64).ravel())
    A_vx = dt_l * (C - gamma_p * I)
    A_vv = (1.0 - dt_l * alpha_p) * I + dt_l * R
    A = np.zeros((D, D))
    A[0::2, 0::2] = I + dt_l * A_vx
    A[0::2, 1::2] = dt_l * A_vv
    A[1::2, 0::2] = A_vx
    A[1::2, 1::2] = A_vv
    b = np.zeros(D)
    b[0::2] = dt_l * dt_l * c0
    b[1::2] = dt_l * c0
    return A, b


def _collapse(A, b, k):
    M = np.eye(A.shape[0])
    d = np.zeros(A.shape[0])
    for _ in range(k):
        M = A @ M
        d = A @ d + b
    return M, d


def _augment_pad(M, d):
    """[DPAD, DPAD] fp64 with homogeneous (bias) row at index D."""
    Mp = np.zeros((DPAD, DPAD))
    Mp[:D, :D] = M
    Mp[:D, D] = d
    Mp[D, D] = 1.0
    return Mp


def _mt_host(Mp, np_dtype=np.float32):
    """lhsT layout: mt[p, kc, m] = Mp[m, kc*128+p]."""
    return np.ascontiguousarray(
        Mp.T.reshape(NK, 128, DPAD).transpose(1, 0, 2)).astype(np_dtype)


# ---------------------------------------------------------------- device IR

_prog_cache = {}


def _build_program(T):
    """Sequential fp32 recurrence: T outer steps, one matmul group per chunk."""
    key = ("v1", T)
    if key in _prog_cache:
        return _prog_cache[key]

    nc = bacc.Bacc("TRN2")
    f32 = mybir.dt.float32
    mt_d = nc.dram_tensor("mt", [128, NK, DPAD], f32, kind="ExternalInput")
    s0_d = nc.dram_tensor("s0", [128, NK, BLOC], f32, kind="ExternalInput")
    y_d = nc.dram_tensor("y", [T, D, BLOC], f32, kind="ExternalOutput")

    with tile.TileContext(nc) as tc, ExitStack() as ctx:
        const = ctx.enter_context(tc.tile_pool(name="const", bufs=1))
        state = ctx.enter_context(tc.tile_pool(name="state", bufs=2))
        psum = ctx.enter_context(tc.tile_pool(name="psum", bufs=4, space="PSUM"))

        mt_sb = const.tile([128, NK, DPAD], f32)
        nc.sync.dma_start(mt_sb[:], mt_d[:])
        s_cur = state.tile([128, NK, BLOC], f32, tag="st")
        nc.sync.dma_start(s_cur[:], s0_d[:])
        # Collapse the many DMA-queue completion semaphores into one barrier
        # so the first matmuls don't exceed the per-instruction wait limit.
        tc.strict_bb_all_engine_barrier()

        for t in range(T):
            s_next = state.tile([128, NK, BLOC], f32, tag="st")
            for mc in range(NK):
                ps = psum.tile([128, BLOC], mybir.dt.float32, tag="ps")
                for kc in range(NK):
                    nc.tensor.matmul(
                        ps,
                        mt_sb[:, kc, mc * 128:(mc + 1) * 128],
                        s_cur[:, kc, :],
                        start=(kc == 0), stop=(kc == NK - 1))
                nc.vector.tensor_copy(s_next[:, mc, :], ps)
                if mc < NK - 1:
                    nc.sync.dma_start(y_d[t, mc * 128:(mc + 1) * 128, :],
                                      s_next[:, mc, :])
                else:
                    nc.sync.dma_start(y_d[t, 8 * 128:D, :],
                                      s_next[:D - 8 * 128, mc, :])
            s_cur = s_next

    nc.finalize()
    _prog_cache[key] = nc
    return nc


def _build_program_chained(T, mm_dt=None):
    """4 interleaved chains (t mod 4) so the PE free dim is 512, where
    fp32r streams 1 cycle/row instead of fp32's 4.

    Ramp (on device): s1 = M s0 ; [s2|s3] = M^2 [s0|s1].
    Steady: U_r = M^4 U_{r-1} with U holding 4 states side by side.
    Requires T >= 4."""
    mm_dt = mm_dt or mybir.dt.float32r
    key = ("v2", T, mm_dt)
    if key in _prog_cache:
        return _prog_cache[key]

    q_full = (T - 3) // 4            # steady rounds: r=1..q_full -> t=4r..4r+3
    tr = T - (4 * q_full + 3)        # 0..3 tail states

    nc = bacc.Bacc("TRN2")
    f32 = mybir.dt.float32
    mt1_d = nc.dram_tensor("mt1", [128, NK, DPAD], mm_dt, kind="ExternalInput")
    mt2_d = nc.dram_tensor("mt2", [128, NK, DPAD], mm_dt, kind="ExternalInput")
    mt4_d = nc.dram_tensor("mt4", [128, NK, DPAD], mm_dt, kind="ExternalInput")
    s0_d = nc.dram_tensor("s0", [128, NK, BLOC], mm_dt, kind="ExternalInput")
    y_d = nc.dram_tensor("y", [T, D, BLOC], f32, kind="ExternalOutput")

    with tile.TileContext(nc) as tc, ExitStack() as ctx:
        const = ctx.enter_context(tc.tile_pool(name="const", bufs=1))
        state = ctx.enter_context(tc.tile_pool(name="state", bufs=2))
        psum = ctx.enter_context(tc.tile_pool(name="psum", bufs=4, space="PSUM"))

        u_cur = state.tile([128, NK, 4 * BLOC], mm_dt, tag="st")
        nc.sync.dma_start(u_cur[:, :, 0:BLOC], s0_d[:])
        mt1_sb = const.tile([128, NK, DPAD], mm_dt)
        mt2_sb = const.tile([128, NK, DPAD], mm_dt)
        mt4_sb = const.tile([128, NK, DPAD], mm_dt)
        nc.sync.dma_start(mt1_sb[:], mt1_d[:])
        nc.sync.dma_start(mt2_sb[:], mt2_d[:])
        nc.sync.dma_start(mt4_sb[:], mt4_d[:])

        def mm(ps, mt_sb, kc, mc, rhs):
            nc.tensor.matmul(
                ps,
                mt_sb[:, kc, mc * 128:(mc + 1) * 128],
                rhs,
                start=(kc == 0), stop=(kc == NK - 1))

        def emit(t, mc, src_cols):
            # state t (1-based) lands at y_d[t-1]; bytes of f32r are f32
            src_cols = src_cols.bitcast(f32)
            if mc < NK - 1:
                nc.sync.dma_start(y_d[t - 1, mc * 128:(mc + 1) * 128, :], src_cols)
            else:
                nc.sync.dma_start(y_d[t - 1, 8 * 128:D, :], src_cols[:D - 8 * 128, :])

        # ramp 1: s1 -> u cols [1B:2B)
        for mc in range(NK):
            ps = psum.tile([128, BLOC], f32, tag="ps")
            for kc in range(NK):
                mm(ps, mt1_sb, kc, mc, u_cur[:, kc, 0:BLOC])
            nc.vector.tensor_copy(u_cur[:, mc, BLOC:2 * BLOC], ps)
            emit(1, mc, u_cur[:, mc, BLOC:2 * BLOC])
        # ramp 2: [s2|s3] -> u cols [2B:4B)
        for mc in range(NK):
            ps = psum.tile([128, 2 * BLOC], f32, tag="ps")
            for kc in range(NK):
                mm(ps, mt2_sb, kc, mc, u_cur[:, kc, 0:2 * BLOC])
            nc.vector.tensor_copy(u_cur[:, mc, 2 * BLOC:4 * BLOC], ps)
            emit(2, mc, u_cur[:, mc, 2 * BLOC:3 * BLOC])
            emit(3, mc, u_cur[:, mc, 3 * BLOC:4 * BLOC])
        # steady
        for r in range(1, q_full + 1):
            u_next = state.tile([128, NK, 4 * BLOC], mm_dt, tag="st")
            for mc in range(NK):
                ps = psum.tile([128, 4 * BLOC], f32, tag="ps")
                for kc in range(NK):
                    mm(ps, mt4_sb, kc, mc, u_cur[:, kc, :])
                nc.vector.tensor_copy(u_next[:, mc, :], ps)
                for c in range(4):
                    emit(4 * r + c, mc, u_next[:, mc, c * BLOC:(c + 1) * BLOC])
            u_cur = u_next
        # tail
        if tr:
            sc = state.tile([128, NK, 4 * BLOC], mm_dt, tag="st")
            for mc in range(NK):
                ps = psum.tile([128, tr * BLOC], f32, tag="ps")
                for kc in range(NK):
                    mm(ps, mt4_sb, kc, mc, u_cur[:, kc, 0:tr * BLOC])
                nc.vector.tensor_copy(sc[:, mc, 0:tr * BLOC], ps)
                for c in range(tr):
                    emit(4 * (q_full + 1) + c, mc, sc[:, mc, c * BLOC:(c + 1) * BLOC])

    nc.finalize()
    _prog_cache[key] = nc
    return nc


# ---------------------------------------------------------------- entry

VARIANT = "v2"
LAST_RESULTS = None


def kernel(**inputs):
    y0 = np.ascontiguousarray(np.asarray(inputs["y0"], np.float32))
    T = int(np.asarray(inputs["num_steps_forward"]))
    B = y0.shape[0]
    assert y0.shape == (B, D) and B == NCORES * BLOC

    out = np.empty((B, T + 1, D), np.float32)
    out[:, 0, :] = y0
    if T == 0:
        return out

    A, b = _build_step_map(
        inputs["W_coupling"], inputs["b_coupling"], inputs["W_resid"],
        inputs["b_resid"], inputs["b_bar"], inputs["dt"], inputs["alpha"],
        inputs["gamma"])
    M, d = _collapse(A, b, 10)
    Mp = _augment_pad(M, d)

    use_v2 = VARIANT == "v2" and T >= 4
    if use_v2:
        Mp2 = Mp @ Mp
        weights = {"mt1": _mt_host(Mp), "mt2": _mt_host(Mp2),
                   "mt4": _mt_host(Mp2 @ Mp2)}
        nc = _build_program_chained(T)
    else:
        weights = {"mt": _mt_host(Mp)}
        nc = _build_program(T)

    # s0 per core: s0[p, kc, b] = s_pad[kc*128+p, b]
    in_maps = []
    for c in range(NCORES):
        sp = np.zeros((DPAD, BLOC), np.float32)
        sp[:D] = y0[c * BLOC:(c + 1) * BLOC].T
        sp[D] = 1.0
        s0c = np.ascontiguousarray(sp.reshape(NK, 128, BLOC).transpose(1, 0, 2))
        in_maps.append({**weights, "s0": s0c})
    global LAST_RESULTS
    LAST_RESULTS = run_bass_kernel_spmd(nc, in_maps, core_ids=list(range(NCORES)))
    for c in range(NCORES):
        yc = LAST_RESULTS.results[c]["y"]            # [T, D, BLOC]
        out[c * BLOC:(c + 1) * BLOC, 1:, :] = yc.transpose(2, 0, 1)
    return out


# revision 11
# speedup vs baseline: 2.4833x; 1.0380x over previous
"""Trainium2 Bass kernel for nn_CoupledOscillatorNetwork.

Math: each inner step of the reference is affine in the flattened state
s = reshape(y, [B, 1058]) (2-channel field on a 23x23 torus):

    v' = dt_l*(C - g*I) x + ((1 - dt_l*a) I + dt_l*R) v + dt_l*c0
    x' = x + dt_l * v'

with C, R the circular 3x3 conv matrices. Ten inner steps therefore
collapse into ONE dense affine map s -> M s + d with M = A^10 computed on
the host in float64 from the (tiny) parameter tensors. The device only
runs the outer recurrence: s_{t+1} = M_aug s_t on an augmented
(homogeneous) state, writing every state to DRAM. Pure data parallelism:
batch 1024 is sharded 128 per NeuronCore across 8 cores.

Device layout (per core), state-major:
  S [1152 x 128]  (state padded 1059->1152 = 9 chunks of 128, batch=128 free)
  per outer step, per output chunk mc: PSUM[128,128] accumulates
  9 matmuls  M_pad^T[kc-chunk, mc-cols] . S[kc-chunk]  ->  copy to next
  state tile + DMA to DRAM.
"""

import numpy as np
from contextlib import ExitStack

import concourse.bass as bass
import concourse.bacc as bacc
import concourse.mybir as mybir
import concourse.tile as tile
from concourse.bass_utils import run_bass_kernel_spmd

SPATIAL = 23
P2 = SPATIAL * SPATIAL          # 529
D = 2 * P2                      # 1058
NK = 9                          # state chunks
DPAD = NK * 128                 # 1152 (state padded incl. homogeneous row 1058)
NCORES = 8
BLOC = 128                      # batch per core

# ---------------------------------------------------------------- host math

def _conv_matrix(W):
    W = np.asarray(W, np.float64).reshape(3, 3)
    idx = np.arange(P2).reshape(SPATIAL, SPATIAL)
    C = np.zeros((P2, P2))
    rows = np.arange(P2)
    for di in range(3):
        for dj in range(3):
            src = np.roll(np.roll(idx, -(di - 1), axis=0), -(dj - 1), axis=1)
            C[rows, src.ravel()] += W[di, dj]
    return C


def _build_step_map(W_coupling, b_coupling, W_resid, b_resid, b_bar, dt, alpha, gamma):
    dt_l = 1.0 / (1.0 + np.exp(-np.float64(dt)))
    gamma_p = max(float(gamma), 0.0)
    alpha_p = max(float(alpha), 0.0)
    C = _conv_matrix(W_coupling)
    R = _conv_matrix(W_resid)
    I = np.eye(P2)
    c0 = (float(np.asarray(b_coupling).ravel()[0])
          + float(np.asarray(b_resid).ravel()[0])
          + np.asarray(b_bar, np.float64).ravel())
    A_vx = dt_l * (C - gamma_p * I)
    A_vv = (1.0 - dt_l * alpha_p) * I + dt_l * R
    A = np.zeros((D, D))
    A[0::2, 0::2] = I + dt_l * A_vx
    A[0::2, 1::2] = dt_l * A_vv
    A[1::2, 0::2] = A_vx
    A[1::2, 1::2] = A_vv
    b = np.zeros(D)
    b[0::2] = dt_l * dt_l * c0
    b[1::2] = dt_l * c0
    return A, b


def _collapse(A, b, k):
    M = np.eye(A.shape[0])
    d = np.zeros(A.shape[0])
    for _ in range(k):
        M = A @ M
        d = A @ d + b
    return M, d


def _augment_pad(M, d):
    """[DPAD, DPAD] fp64 with homogeneous (bias) row at index D."""
    Mp = np.zeros((DPAD, DPAD))
    Mp[:D, :D] = M
    Mp[:D, D] = d
    Mp[D, D] = 1.0
    return Mp


def _mt_host(Mp, np_dtype=np.float32):
    """lhsT layout: mt[p, kc, m] = Mp[m, kc*128+p]."""
    return np.ascontiguousarray(
        Mp.T.reshape(NK, 128, DPAD).transpose(1, 0, 2)).astype(np_dtype)


# ---------------------------------------------------------------- device IR

_prog_cache = {}


def _build_program(T):
    """Sequential fp32 recurrence: T outer steps, one matmul group per chunk."""
    key = ("v1", T)
    if key in _prog_cache:
        return _prog_cache[key]

    nc = bacc.Bacc("TRN2")
    f32 = mybir.dt.float32
    mt_d = nc.dram_tensor("mt", [128, NK, DPAD], f32, kind="ExternalInput")
    s0_d = nc.dram_tensor("s0", [128, NK, BLOC], f32, kind="ExternalInput")
    y_d = nc.dram_tensor("y", [T, D, BLOC], f32, kind="ExternalOutput")

    with tile.TileContext(nc) as tc, ExitStack() as ctx:
        const = ctx.enter_context(tc.tile_pool(name="const", bufs=1))
        state = ctx.enter_context(tc.tile_pool(name="state", bufs=2))
        psum = ctx.enter_context(tc.tile_pool(name="psum", bufs=4, space="PSUM"))

        mt_sb = const.tile([128, NK, DPAD], f32)
        nc.sync.dma_start(mt_sb[:], mt_d[:])
        s_cur = state.tile([128, NK, BLOC], f32, tag="st")
        nc.sync.dma_start(s_cur[:], s0_d[:])
        # Collapse the many DMA-queue completion semaphores into one barrier
        # so the first matmuls don't exceed the per-instruction wait limit.
        tc.strict_bb_all_engine_barrier()

        for t in range(T):
            s_next = state.tile([128, NK, BLOC], f32, tag="st")
            for mc in range(NK):
                ps = psum.tile([128, BLOC], mybir.dt.float32, tag="ps")
                for kc in range(NK):
                    nc.tensor.matmul(
                        ps,
                        mt_sb[:, kc, mc * 128:(mc + 1) * 128],
                        s_cur[:, kc, :],
                        start=(kc == 0), stop=(kc == NK - 1))
                nc.vector.tensor_copy(s_next[:, mc, :], ps)
                if mc < NK - 1:
                    nc.sync.dma_start(y_d[t, mc * 128:(mc + 1) * 128, :],
                                      s_next[:, mc, :])
                else:
                    nc.sync.dma_start(y_d[t, 8 * 128:D, :],
                                      s_next[:D - 8 * 128, mc, :])
            s_cur = s_next

    nc.finalize()
    _prog_cache[key] = nc
    return nc


def _build_program_chained(T, mm_dt=None):
    """4 interleaved chains (t mod 4) so the PE free dim is 512, where
    fp32r streams 1 cycle/row instead of fp32's 4.

    Ramp (on device): s1 = M s0 ; [s2|s3] = M^2 [s0|s1].
    Steady: U_r = M^4 U_{r-1} with U holding 4 states side by side.
    Requires T >= 4."""
    mm_dt = mm_dt or mybir.dt.float32r
    key = ("v2", T, mm_dt)
    if key in _prog_cache:
        return _prog_cache[key]

    q_full = (T - 3) // 4            # steady rounds: r=1..q_full -> t=4r..4r+3
    tr = T - (4 * q_full + 3)        # 0..3 tail states

    nc = bacc.Bacc("TRN2")
    f32 = mybir.dt.float32
    mt1_d = nc.dram_tensor("mt1", [128, NK, DPAD], mm_dt, kind="ExternalInput")
    mt2_d = nc.dram_tensor("mt2", [128, NK, DPAD], mm_dt, kind="ExternalInput")
    mt4_d = nc.dram_tensor("mt4", [128, NK, DPAD], mm_dt, kind="ExternalInput")
    s0_d = nc.dram_tensor("s0", [128, NK, BLOC], mm_dt, kind="ExternalInput")
    y_d = nc.dram_tensor("y", [T, D, BLOC], f32, kind="ExternalOutput")

    with tile.TileContext(nc) as tc, ExitStack() as ctx:
        const = ctx.enter_context(tc.tile_pool(name="const", bufs=1))
        state = ctx.enter_context(tc.tile_pool(name="state", bufs=3))
        psum = ctx.enter_context(tc.tile_pool(name="psum", bufs=6, space="PSUM"))

        u_cur = state.tile([128, NK, 4 * BLOC], mm_dt, tag="st")
        nc.sync.dma_start(u_cur[:, :, 0:BLOC], s0_d[:])
        mt1_sb = const.tile([128, NK, DPAD], mm_dt)
        mt2_sb = const.tile([128, NK, DPAD], mm_dt)
        mt4_sb = const.tile([128, NK, DPAD], mm_dt)
        nc.sync.dma_start(mt1_sb[:], mt1_d[:])
        nc.sync.dma_start(mt2_sb[:], mt2_d[:])
        nc.sync.dma_start(mt4_sb[:], mt4_d[:])

        def mm(ps, mt_sb, kc, mc, rhs):
            nc.tensor.matmul(
                ps,
                mt_sb[:, kc, mc * 128:(mc + 1) * 128],
                rhs,
                start=(kc == 0), stop=(kc == NK - 1))

        def emit(t, mc, src_cols):
            # state t (1-based) lands at y_d[t-1]; bytes of f32r are f32
            src_cols = src_cols.bitcast(f32)
            if mc < NK - 1:
                nc.sync.dma_start(y_d[t - 1, mc * 128:(mc + 1) * 128, :], src_cols)
            else:
                nc.sync.dma_start(y_d[t - 1, 8 * 128:D, :], src_cols[:D - 8 * 128, :])

        # ramp 1: s1 -> u cols [1B:2B)
        for mc in range(NK):
            ps = psum.tile([128, BLOC], f32, tag="ps")
            for kc in range(NK):
                mm(ps, mt1_sb, kc, mc, u_cur[:, kc, 0:BLOC])
            nc.vector.tensor_copy(u_cur[:, mc, BLOC:2 * BLOC], ps)
            emit(1, mc, u_cur[:, mc, BLOC:2 * BLOC])
        # ramp 2: [s2|s3] -> u cols [2B:4B)
        for mc in range(NK):
            ps = psum.tile([128, 2 * BLOC], f32, tag="ps")
            for kc in range(NK):
                mm(ps, mt2_sb, kc, mc, u_cur[:, kc, 0:2 * BLOC])
            nc.vector.tensor_copy(u_cur[:, mc, 2 * BLOC:4 * BLOC], ps)
            emit(2, mc, u_cur[:, mc, 2 * BLOC:3 * BLOC])
            emit(3, mc, u_cur[:, mc, 3 * BLOC:4 * BLOC])
        # steady
        for r in range(1, q_full + 1):
            u_next = state.tile([128, NK, 4 * BLOC], mm_dt, tag="st")
            for mc in range(NK):
                ps = psum.tile([128, 4 * BLOC], f32, tag="ps")
                for kc in range(NK):
                    mm(ps, mt4_sb, kc, mc, u_cur[:, kc, :])
                nc.vector.tensor_copy(u_next[:, mc, :], ps)
                for c in range(4):
                    emit(4 * r + c, mc, u_next[:, mc, c * BLOC:(c + 1) * BLOC])
            u_cur = u_next
        # tail
        if tr:
            sc = state.tile([128, NK, 4 * BLOC], mm_dt, tag="st")
            for mc in range(NK):
                ps = psum.tile([128, tr * BLOC], f32, tag="ps")
                for kc in range(NK):
                    mm(ps, mt4_sb, kc, mc, u_cur[:, kc, 0:tr * BLOC])
                nc.vector.tensor_copy(sc[:, mc, 0:tr * BLOC], ps)
                for c in range(tr):
                    emit(4 * (q_full + 1) + c, mc, sc[:, mc, c * BLOC:(c + 1) * BLOC])

    nc.finalize()
    _prog_cache[key] = nc
    return nc


# ---------------------------------------------------------------- entry

VARIANT = "v2"
LAST_RESULTS = None


def kernel(**inputs):
    y0 = np.ascontiguousarray(np.asarray(inputs["y0"], np.float32))
    T = int(np.asarray(inputs["num_steps_forward"]))
    B = y0.shape[0]
    assert y0.shape == (B, D) and B == NCORES * BLOC

    out = np.empty((B, T + 1, D), np.float32)
    out[:, 0, :] = y0
    if T == 0:
        return out

    A, b = _build_step_map(
        inputs["W_coupling"], inputs["b_coupling"], inputs["W_resid"],
        inputs["b_resid"], inputs["b_bar"], inputs["dt"], inputs["alpha"],
        inputs["gamma"])
    M, d = _collapse(A, b, 10)
    Mp = _augment_pad(M, d)

    use_v2 = VARIANT == "v2" and T >= 4
    if use_v2:
        Mp2 = Mp @ Mp
        weights = {"mt1": _mt_host(Mp), "mt2": _mt_host(Mp2),
                   "mt4": _mt_host(Mp2 @ Mp2)}
        nc = _build_program_chained(T)
    else:
        weights = {"mt": _mt_host(Mp)}
        nc = _build_program(T)

    # s0 per core: s0[p, kc, b] = s_pad[kc*128+p, b]
    in_maps = []
    for c in range(NCORES):
        sp = np.zeros((DPAD, BLOC), np.float32)
        sp[:D] = y0[c * BLOC:(c + 1) * BLOC].T
        sp[D] = 1.0
        s0c = np.ascontiguousarray(sp.reshape(NK, 128, BLOC).transpose(1, 0, 2))
        in_maps.append({**weights, "s0": s0c})
    global LAST_RESULTS
    LAST_RESULTS = run_bass_kernel_spmd(nc, in_maps, core_ids=list(range(NCORES)))
    for c in range(NCORES):
        yc = LAST_RESULTS.results[c]["y"]            # [T, D, BLOC]
        out[c * BLOC:(c + 1) * BLOC, 1:, :] = yc.transpose(2, 0, 1)
    return out
